# revision 26
# baseline (speedup 1.0000x reference)
"""Multi-head dense attention (no softmax) on 8 Trainium2 NeuronCores.

Math (per batch b, head h with head_dim d=64):
    q   = x @ W^T                      # [S, H] projection
    out_h = (q_h x_h^T) x_h            # naive: O(S^2 d) with an SxS temp
          = q_h (x_h^T x_h)            # reassociated: Gram matrix G_h [d, d]
The reassociation is exact (same sum, different order) and collapses the
FLOPs ~5x while removing the SxS intermediate entirely.

Sharding: core c handles batch b = c//2 and head-group hg = c%2 (8 heads,
512 output columns). Cores are fully independent (no collectives).

v9 (v1 61.9us, v5 54.0, v6 53.7). The first ~20us is wire-limited (one
DMA ring, ~8.3us to first byte after the fixed framework preamble), so
v9 restructures the stream around the wire:
  - wT[kt] + xT[sc0,kt] are packed into ONE interleaved uint8 "bundle"
    param (cell kt = 1024B wT f16 | 512B xT e3m4 per partition row). One
    trigger per kt-group moves both operands of the next projection
    k-step: no paired-trigger dribble, 1.5-3KiB contiguous lines, and
    matmuls read the cells through bitcast views of a single SBUF tile.
  - xn is re-blocked column-pair-major, shipped 2 pairs before / 2 after
    the bundle, and the Gram runs PER PAIR inside the wire-limited
    window (pairs 0-1 right after warmup, pairs 2-3 after proj1) -
    its ~2.5us of PE time disappears from the dense back half.
  - Everything else keeps the v5/v6 structure: fp8 e4m3 DoubleRow Gram,
    e3m4 xT (mixed f16 x f8e3 matmul verified on HW; e4m3 anywhere in
    the projection FAILS the 2e-2 gate at 2.5-3.5e-2 - simulated), f16
    wT/q/out, drains split in halves across Vector+Activation, psq0
    double-buffered with (1,2,3,0)/(0,1,2,3) drain orders, pso ring of
    3 banks shared by warmup/gram/out-stage, ot staging ring of 4, one
    batched store per s-chunk (per-mt for the last), warmup chain + early
    ACT-table preload.

Tensor order: warmup | gram01 | proj0 | proj1 | gram23 | out0 | proj2 |
out1 | out2 | proj3 | out3.

Device layout per core (all partition-outer):
    bundle [128, KT*1536] u8   cell kt = wT[kt] f16 (1024B) | xT[0,kt]
                               e3m4 (512B)
    xT   [3*128, KT*512] f8e3  s-chunks 1-3, row sc*128+p = kt chunks
    xn   [128, MT*ST*128] f8e4 column-pair-major: [pair][st][128]
    outB [128, SC*MT*512] f16  row p = out^T chunks; host reassembles
"""

import numpy as np

B, S, H = 4, 2048, 1024
N_HEADS = 16
HD = H // N_HEADS  # 64
N_CORES = 8
MG = H // 2        # 512 output columns per core
P = 128
KT = H // P        # 8 k-tiles
ST = S // P        # 16 s-tiles
MT = MG // P       # 4 m-tiles == head pairs
SC = S // 512      # 4 s-chunks
W_SCALE = 1024.0
N_WARMUP = 8
CELL = 1536        # bundle cell bytes: 1024 (wT f16) + 512 (xT e3m4)

_NC_CACHE = {}


def _build_nc():
    import concourse.mybir as mybir
    from concourse import bacc
    from concourse.tile import TileContext

    f32 = mybir.dt.float32
    f16 = mybir.dt.float16
    f8e4 = mybir.dt.float8e4
    f8e3 = mybir.dt.float8e3
    u8 = mybir.dt.uint8
    DR = mybir.MatmulPerfMode.DoubleRow

    nc = bacc.Bacc()
    bun_d = nc.declare_dram_parameter("bundle", [P, KT * CELL], u8, isOutput=False)
    xT_d = nc.declare_dram_parameter(
        "xT", [(SC - 1) * P, KT * 512], f8e3, isOutput=False
    )
    xn_d = nc.declare_dram_parameter("xn", [P, ST * MG], f8e4, isOutput=False)
    outB_d = nc.declare_dram_parameter(
        "outB", [P, SC * MT * 512], f16, isOutput=True
    )

    bun_t = bun_d.rearrange("p (kt b) -> p kt b", kt=KT)
    xT_t = xT_d.rearrange("(sc p) (kt n) -> p sc kt n", sc=SC - 1, kt=KT)
    xn_t = xn_d.rearrange("p (st m) -> p st m", st=ST)
    outB_t = outB_d.rearrange("p (sc mt n) -> p sc mt n", sc=SC, mt=MT)

    with TileContext(nc) as tc:
        with (
            tc.tile_pool(name="big", bufs=1) as big,
            tc.tile_pool(name="gp", bufs=1) as gpool,
            tc.tile_pool(name="stage", bufs=4) as stage,
            tc.tile_pool(name="ps_q0", bufs=2, space="PSUM") as ps_q0,
            tc.tile_pool(name="ps_q", bufs=1, space="PSUM") as ps_q,
            tc.tile_pool(name="ps_o", bufs=3, space="PSUM") as ps_o,
        ):
            bun_sb = big.tile([P, KT, CELL], u8, tag="bundle")
            xT_sb = big.tile([P, SC - 1, KT, 512], f8e3, tag="xT")
            xn_sb = big.tile([P, ST, MG], f8e4, tag="xn")
            q_sb = big.tile([P, MT, S], f16, tag="q")
            # Bitcast views into the bundle cells.
            wv = [bun_sb[:, kt, 0:1024].bitcast(f16) for kt in range(KT)]
            xv0 = [bun_sb[:, kt, 1024:CELL].bitcast(f8e3) for kt in range(KT)]

            # ---- Warmup: one back-to-back accumulation chain (same psum
            # tile, same engine => no semaphores) spins the PE p-state up
            # during the initial DMA latency window. The scalar-engine copy
            # forces the lazy ACT_TABLE_LOAD into this idle window too.
            wu_sb = gpool.tile([P, 512], f16, tag="wu", name="wu_sb")
            nc.vector.memset(wu_sb, 0.0)
            nc.scalar.copy(out=wu_sb[:, 256:264], in_=wu_sb[:, 0:8])
            gbd = []
            for p_i in range(MT):
                g = gpool.tile([P, P], f16, tag=f"g{p_i}", name=f"g{p_i}")
                nc.vector.memset(g, 0.0)
                gbd.append(g)
            wu_ps = ps_o.tile([P, 256], f32, tag="pso", name="wu_ps")
            for i in range(N_WARMUP):
                nc.tensor.matmul(
                    wu_ps,
                    lhsT=wu_sb[:, 0:P],
                    rhs=wu_sb[:, 0:256],
                    start=(i == 0),
                    stop=(i == N_WARMUP - 1),
                )

            # ---- Input DMA ring (Sync engine), exact consumption order.
            for a, z in ((0, 1), (1, 3), (3, 5), (5, 7), (7, 8)):
                nc.sync.dma_start(out=bun_sb[:, a:z], in_=bun_t[:, a:z])
            nc.sync.dma_start(out=xT_sb[:, 0, 0:4], in_=xT_t[:, 0, 0:4])
            nc.sync.dma_start(out=xT_sb[:, 0, 4:8], in_=xT_t[:, 0, 4:8])
            nc.sync.dma_start(out=xn_sb[:, 0:8], in_=xn_t[:, 0:8])
            nc.sync.dma_start(out=xn_sb[:, 8:16], in_=xn_t[:, 8:16])
            nc.sync.dma_start(out=xT_sb[:, 1], in_=xT_t[:, 1])
            nc.sync.dma_start(out=xT_sb[:, 2], in_=xT_t[:, 2])

            def proj(sc, drain_order):
                psqs = [
                    (ps_q0 if mt == 0 else ps_q).tile(
                        [P, 512], f32, tag=f"psq{mt}", name=f"psq{sc}_{mt}"
                    )
                    for mt in range(MT)
                ]
                for kt in range(KT):
                    rhs = xv0[kt] if sc == 0 else xT_sb[:, sc - 1, kt]
                    for mt in range(MT):
                        nc.tensor.matmul(
                            psqs[mt],
                            lhsT=wv[kt][:, mt * P:(mt + 1) * P],
                            rhs=rhs,
                            start=(kt == 0),
                            stop=(kt == KT - 1),
                        )
                for mt in drain_order:
                    lo = q_sb[:, mt, sc * 512:sc * 512 + 256]
                    hi = q_sb[:, mt, sc * 512 + 256:(sc + 1) * 512]
                    nc.vector.tensor_copy(out=lo, in_=psqs[mt][:, 0:256])
                    nc.scalar.copy(out=hi, in_=psqs[mt][:, 256:512])

            def gram(p_i):
                psg = ps_o.tile([P, P], f32, tag="pso", name=f"psg{p_i}")
                xp = xn_sb[:, :, p_i * P:(p_i + 1) * P]
                for i in range(ST // 2):
                    nc.tensor.matmul(
                        psg,
                        lhsT=xp[:, 2 * i:2 * i + 2],
                        rhs=xp[:, 2 * i:2 * i + 2],
                        start=(i == 0),
                        stop=(i == ST // 2 - 1),
                        perf_mode=DR,
                    )
                nc.vector.tensor_scalar_mul(
                    out=gbd[p_i][0:HD, 0:HD],
                    in0=psg[0:HD, 0:HD],
                    scalar1=1.0 / W_SCALE,
                )
                nc.scalar.mul(
                    gbd[p_i][HD:P, HD:P], psg[HD:P, HD:P], 1.0 / W_SCALE
                )

            def out_stage(sc):
                ot = stage.tile([P, MT, 512], f16, tag="ot", name=f"ot{sc}")
                for mt in range(MT):
                    pso = ps_o.tile([P, 512], f32, tag="pso", name=f"pso{sc}_{mt}")
                    nc.tensor.matmul(
                        pso,
                        lhsT=gbd[mt],
                        rhs=q_sb[:, mt, sc * 512:(sc + 1) * 512],
                        start=True,
                        stop=True,
                    )
                    nc.vector.tensor_copy(out=ot[:, mt, 0:256], in_=pso[:, 0:256])
                    nc.scalar.copy(out=ot[:, mt, 256:512], in_=pso[:, 256:512])
                    if sc == SC - 1:
                        # Per-mt stores so the last chunk's wire time
                        # overlaps the remaining drains.
                        nc.gpsimd.dma_start(
                            out=outB_t[:, sc, mt:mt + 1], in_=ot[:, mt:mt + 1]
                        )
                if sc != SC - 1:
                    nc.gpsimd.dma_start(out=outB_t[:, sc], in_=ot)

            proj(0, (1, 2, 3, 0))
            proj(1, (1, 2, 3, 0))
            for p_i in range(MT):
                gram(p_i)
            out_stage(0)
            proj(2, (0, 1, 2, 3))
            out_stage(1)
            out_stage(2)
            proj(3, (0, 1, 2, 3))
            out_stage(3)
    nc.compile()
    return nc


def _get_nc():
    if "nc" not in _NC_CACHE:
        _NC_CACHE["nc"] = _build_nc()
    return _NC_CACHE["nc"]


def make_in_maps(hidden_states, queries_weight):
    import ml_dtypes

    f8e4 = ml_dtypes.float8_e4m3
    f8e3 = ml_dtypes.float8_e3m4
    hs = np.ascontiguousarray(np.asarray(hidden_states, dtype=np.float32))
    w = np.ascontiguousarray(np.asarray(queries_weight, dtype=np.float32))
    in_maps = []
    for c in range(N_CORES):
        b, hg = divmod(c, 2)
        xb = hs[b]
        xT = np.ascontiguousarray(xb.T)  # [1024, 2048]
        xTq = xT.reshape(KT, P, SC, 512).astype(f8e3)  # (kt, p, sc, n)
        wTq = (
            (w[hg * MG:(hg + 1) * MG, :].T * W_SCALE)
            .reshape(KT, P, MG).astype(np.float16)
        )
        # bundle cell kt = [wT[kt] 1024B | xT[sc0,kt] 512B] per partition
        cells = []
        for kt in range(KT):
            cells.append(np.ascontiguousarray(wTq[kt]).view(np.uint8))
            cells.append(np.ascontiguousarray(xTq[kt, :, 0, :]).view(np.uint8))
        in_maps.append({
            "bundle": np.ascontiguousarray(np.concatenate(cells, axis=1)),
            # s-chunks 1-3: row sc*128+p holds kt-major chunks
            "xT": np.ascontiguousarray(
                xTq[:, :, 1:, :].transpose(2, 1, 0, 3).reshape(
                    (SC - 1) * P, KT * 512
                )
            ),
            # row p holds st-major chunks: [P, ST*MG]
            "xn": np.ascontiguousarray(
                xb[:, hg * MG:(hg + 1) * MG]
                .reshape(ST, P, MG).transpose(1, 0, 2).reshape(P, ST * MG)
            ).astype(f8e4),
        })
    return in_maps


def assemble_output(results):
    out = np.empty((B, S, H), dtype=np.float32)
    for c in range(N_CORES):
        b, hg = divmod(c, 2)
        r = np.asarray(results[c]["outB"])  # [P, SC*MT*512] f16
        out[b, :, hg * MG:(hg + 1) * MG] = (
            r.reshape(P, SC, MT, 512).transpose(1, 3, 2, 0).reshape(S, MG)
        ).astype(np.float32)
    return out


def kernel(hidden_states, queries_weight):
    from concourse.bass_utils import run_bass_kernel_spmd

    in_maps = make_in_maps(hidden_states, queries_weight)
    res = run_bass_kernel_spmd(
        _get_nc(), in_maps, core_ids=list(range(N_CORES))
    ).results
    return assemble_output(res)


if __name__ == "__main__":
    x = np.random.randn(B, S, H).astype(np.float32)
    w = np.random.randn(H, H).astype(np.float32) * 1e-4
    out = kernel(x, w)
    print(out.shape, out.dtype)


# revision 30
# speedup vs baseline: 1.0516x; 1.0516x over previous
"""Multi-head dense attention (no softmax) on 8 Trainium2 NeuronCores.

Math (per batch b, head h with head_dim d=64):
    q   = x @ W^T                      # [S, H] projection
    out_h = (q_h x_h^T) x_h            # naive: O(S^2 d) with an SxS temp
          = q_h (x_h^T x_h)            # reassociated: Gram matrix G_h [d, d]
The reassociation is exact (same sum, different order) and collapses the
FLOPs ~5x while removing the SxS intermediate entirely.

Sharding: core c handles batch b = c//2 and head-group hg = c%2 (8 heads,
512 output columns). Cores are fully independent (no collectives).

v9 (v1 61.9us, v5 54.0, v6 53.7). The first ~20us is wire-limited (one
DMA ring, ~8.3us to first byte after the fixed framework preamble), so
v9 restructures the stream around the wire:
  - wT[kt] + xT[sc0,kt] are packed into ONE interleaved uint8 "bundle"
    param (cell kt = 1024B wT f16 | 512B xT e3m4 per partition row). One
    trigger per kt-group moves both operands of the next projection
    k-step: no paired-trigger dribble, 1.5-3KiB contiguous lines, and
    matmuls read the cells through bitcast views of a single SBUF tile.
  - xn is re-blocked column-pair-major, shipped 2 pairs before / 2 after
    the bundle, and the Gram runs PER PAIR inside the wire-limited
    window (pairs 0-1 right after warmup, pairs 2-3 after proj1) -
    its ~2.5us of PE time disappears from the dense back half.
  - Everything else keeps the v5/v6 structure: fp8 e4m3 DoubleRow Gram,
    e3m4 xT (mixed f16 x f8e3 matmul verified on HW; e4m3 anywhere in
    the projection FAILS the 2e-2 gate at 2.5-3.5e-2 - simulated), f16
    wT/q/out, drains split in halves across Vector+Activation, psq0
    double-buffered with (1,2,3,0)/(0,1,2,3) drain orders, pso ring of
    3 banks shared by warmup/gram/out-stage, ot staging ring of 4, one
    batched store per s-chunk (per-mt for the last), warmup chain + early
    ACT-table preload.

Tensor order: warmup | gram01 | proj0 | proj1 | gram23 | out0 | proj2 |
out1 | out2 | proj3 | out3.

Device layout per core (all partition-outer):
    bundle [128, KT*1536] u8   cell kt = wT[kt] f16 (1024B) | xT[0,kt]
                               e3m4 (512B)
    xT   [3*128, KT*512] f8e3  s-chunks 1-3, row sc*128+p = kt chunks
    xn   [128, MT*ST*128] f8e4 column-pair-major: [pair][st][128]
    outB [128, SC*MT*512] f16  row p = out^T chunks; host reassembles
"""

import numpy as np

B, S, H = 4, 2048, 1024
N_HEADS = 16
HD = H // N_HEADS  # 64
N_CORES = 8
MG = H // 2        # 512 output columns per core
P = 128
KT = H // P        # 8 k-tiles
ST = S // P        # 16 s-tiles
MT = MG // P       # 4 m-tiles == head pairs
SC = S // 512      # 4 s-chunks
W_SCALE = 1024.0
N_WARMUP = 8
CELL = 1536        # bundle cell bytes: 1024 (wT f16) + 512 (xT e3m4)

_NC_CACHE = {}


def _build_nc():
    import concourse.mybir as mybir
    from concourse import bacc
    from concourse.tile import TileContext

    f32 = mybir.dt.float32
    f16 = mybir.dt.float16
    f8e4 = mybir.dt.float8e4
    f8e3 = mybir.dt.float8e3
    u8 = mybir.dt.uint8
    DR = mybir.MatmulPerfMode.DoubleRow

    nc = bacc.Bacc()
    bun_d = nc.declare_dram_parameter("bundle", [P, KT * CELL], u8, isOutput=False)
    xT_d = nc.declare_dram_parameter(
        "xT", [(SC - 1) * P, KT * 512], f8e3, isOutput=False
    )
    xn_d = nc.declare_dram_parameter("xn", [P, ST * MG], f8e4, isOutput=False)
    outB_d = nc.declare_dram_parameter(
        "outB", [P, SC * MT * 512], f16, isOutput=True
    )

    bun_t = bun_d.rearrange("p (kt b) -> p kt b", kt=KT)
    xT_t = xT_d.rearrange("(sc p) (kt n) -> p sc kt n", sc=SC - 1, kt=KT)
    xn_t = xn_d.rearrange("p (st m) -> p st m", st=ST)
    outB_t = outB_d.rearrange("p (sc mt n) -> p sc mt n", sc=SC, mt=MT)

    with TileContext(nc) as tc:
        with (
            tc.tile_pool(name="big", bufs=1) as big,
            tc.tile_pool(name="gp", bufs=1) as gpool,
            tc.tile_pool(name="stage", bufs=4) as stage,
            tc.tile_pool(name="ps_q0", bufs=2, space="PSUM") as ps_q0,
            tc.tile_pool(name="ps_q", bufs=1, space="PSUM") as ps_q,
            tc.tile_pool(name="ps_o", bufs=3, space="PSUM") as ps_o,
        ):
            # One bundle tile per DMA trigger group: dependency tracking
            # does not see sub-tile ranges through bitcast views, so a
            # single tile would make the first matmul wait for the LAST
            # trigger. Separate tiles restore per-chunk streaming.
            BGROUPS = ((0, 1), (1, 3), (3, 5), (5, 7), (7, 8))
            bun_sbs = [
                big.tile([P, (z - a) * CELL], u8, tag=f"bun{a}", name=f"bun{a}")
                for a, z in BGROUPS
            ]
            xT_sb = big.tile([P, SC - 1, KT, 512], f8e3, tag="xT")
            xn_sb = big.tile([P, ST, MG], f8e4, tag="xn")
            q_sb = big.tile([P, MT, S], f16, tag="q")
            # Bitcast views into the bundle cells.
            wv, xv0 = [], []
            for (a, z), t in zip(BGROUPS, bun_sbs):
                for j in range(z - a):
                    wv.append(t[:, j * CELL:j * CELL + 1024].bitcast(f16))
                    xv0.append(
                        t[:, j * CELL + 1024:(j + 1) * CELL].bitcast(f8e3)
                    )

            # ---- Warmup: one back-to-back accumulation chain (same psum
            # tile, same engine => no semaphores) spins the PE p-state up
            # during the initial DMA latency window. The scalar-engine copy
            # forces the lazy ACT_TABLE_LOAD into this idle window too.
            wu_sb = gpool.tile([P, 512], f16, tag="wu", name="wu_sb")
            nc.vector.memset(wu_sb, 0.0)
            nc.scalar.copy(out=wu_sb[:, 256:264], in_=wu_sb[:, 0:8])
            gbd = []
            for p_i in range(MT):
                g = gpool.tile([P, P], f16, tag=f"g{p_i}", name=f"g{p_i}")
                nc.vector.memset(g, 0.0)
                gbd.append(g)
            wu_ps = ps_o.tile([P, 256], f32, tag="pso", name="wu_ps")
            for i in range(N_WARMUP):
                nc.tensor.matmul(
                    wu_ps,
                    lhsT=wu_sb[:, 0:P],
                    rhs=wu_sb[:, 0:256],
                    start=(i == 0),
                    stop=(i == N_WARMUP - 1),
                )

            # ---- Input DMA ring (Sync engine), exact consumption order.
            for (a, z), t in zip(BGROUPS, bun_sbs):
                nc.sync.dma_start(out=t, in_=bun_d[:, a * CELL:z * CELL])
            nc.sync.dma_start(out=xT_sb[:, 0, 0:4], in_=xT_t[:, 0, 0:4])
            nc.sync.dma_start(out=xT_sb[:, 0, 4:8], in_=xT_t[:, 0, 4:8])
            nc.sync.dma_start(out=xn_sb[:, 0:8], in_=xn_t[:, 0:8])
            nc.sync.dma_start(out=xn_sb[:, 8:16], in_=xn_t[:, 8:16])
            nc.sync.dma_start(out=xT_sb[:, 1], in_=xT_t[:, 1])
            nc.sync.dma_start(out=xT_sb[:, 2], in_=xT_t[:, 2])

            def proj(sc, drain_order):
                psqs = [
                    (ps_q0 if mt == 0 else ps_q).tile(
                        [P, 512], f32, tag=f"psq{mt}", name=f"psq{sc}_{mt}"
                    )
                    for mt in range(MT)
                ]
                for kt in range(KT):
                    rhs = xv0[kt] if sc == 0 else xT_sb[:, sc - 1, kt]
                    for mt in range(MT):
                        nc.tensor.matmul(
                            psqs[mt],
                            lhsT=wv[kt][:, mt * P:(mt + 1) * P],
                            rhs=rhs,
                            start=(kt == 0),
                            stop=(kt == KT - 1),
                        )
                for mt in drain_order:
                    lo = q_sb[:, mt, sc * 512:sc * 512 + 256]
                    hi = q_sb[:, mt, sc * 512 + 256:(sc + 1) * 512]
                    nc.vector.tensor_copy(out=lo, in_=psqs[mt][:, 0:256])
                    nc.scalar.copy(out=hi, in_=psqs[mt][:, 256:512])

            def gram(p_i):
                psg = ps_o.tile([P, P], f32, tag="pso", name=f"psg{p_i}")
                xp = xn_sb[:, :, p_i * P:(p_i + 1) * P]
                for i in range(ST // 2):
                    nc.tensor.matmul(
                        psg,
                        lhsT=xp[:, 2 * i:2 * i + 2],
                        rhs=xp[:, 2 * i:2 * i + 2],
                        start=(i == 0),
                        stop=(i == ST // 2 - 1),
                        perf_mode=DR,
                    )
                nc.vector.tensor_scalar_mul(
                    out=gbd[p_i][0:HD, 0:HD],
                    in0=psg[0:HD, 0:HD],
                    scalar1=1.0 / W_SCALE,
                )
                nc.scalar.mul(
                    gbd[p_i][HD:P, HD:P], psg[HD:P, HD:P], 1.0 / W_SCALE
                )

            def out_stage(sc):
                ot = stage.tile([P, MT, 512], f16, tag="ot", name=f"ot{sc}")
                for mt in range(MT):
                    pso = ps_o.tile([P, 512], f32, tag="pso", name=f"pso{sc}_{mt}")
                    nc.tensor.matmul(
                        pso,
                        lhsT=gbd[mt],
                        rhs=q_sb[:, mt, sc * 512:(sc + 1) * 512],
                        start=True,
                        stop=True,
                    )
                    nc.vector.tensor_copy(out=ot[:, mt, 0:256], in_=pso[:, 0:256])
                    nc.scalar.copy(out=ot[:, mt, 256:512], in_=pso[:, 256:512])
                    if sc == SC - 1:
                        # Per-mt stores so the last chunk's wire time
                        # overlaps the remaining drains.
                        nc.gpsimd.dma_start(
                            out=outB_t[:, sc, mt:mt + 1], in_=ot[:, mt:mt + 1]
                        )
                if sc != SC - 1:
                    nc.gpsimd.dma_start(out=outB_t[:, sc], in_=ot)

            proj(0, (1, 2, 3, 0))
            proj(1, (1, 2, 3, 0))
            for p_i in range(MT):
                gram(p_i)
            out_stage(0)
            proj(2, (0, 1, 2, 3))
            out_stage(1)
            out_stage(2)
            proj(3, (0, 1, 2, 3))
            out_stage(3)
    nc.compile()
    return nc


def _get_nc():
    if "nc" not in _NC_CACHE:
        _NC_CACHE["nc"] = _build_nc()
    return _NC_CACHE["nc"]


def make_in_maps(hidden_states, queries_weight):
    import ml_dtypes

    f8e4 = ml_dtypes.float8_e4m3
    f8e3 = ml_dtypes.float8_e3m4
    hs = np.ascontiguousarray(np.asarray(hidden_states, dtype=np.float32))
    w = np.ascontiguousarray(np.asarray(queries_weight, dtype=np.float32))
    in_maps = []
    for c in range(N_CORES):
        b, hg = divmod(c, 2)
        xb = hs[b]
        xT = np.ascontiguousarray(xb.T)  # [1024, 2048]
        xTq = xT.reshape(KT, P, SC, 512).astype(f8e3)  # (kt, p, sc, n)
        wTq = (
            (w[hg * MG:(hg + 1) * MG, :].T * W_SCALE)
            .reshape(KT, P, MG).astype(np.float16)
        )
        # bundle cell kt = [wT[kt] 1024B | xT[sc0,kt] 512B] per partition
        cells = []
        for kt in range(KT):
            cells.append(np.ascontiguousarray(wTq[kt]).view(np.uint8))
            cells.append(np.ascontiguousarray(xTq[kt, :, 0, :]).view(np.uint8))
        in_maps.append({
            "bundle": np.ascontiguousarray(np.concatenate(cells, axis=1)),
            # s-chunks 1-3: row sc*128+p holds kt-major chunks
            "xT": np.ascontiguousarray(
                xTq[:, :, 1:, :].transpose(2, 1, 0, 3).reshape(
                    (SC - 1) * P, KT * 512
                )
            ),
            # row p holds st-major chunks: [P, ST*MG]
            "xn": np.ascontiguousarray(
                xb[:, hg * MG:(hg + 1) * MG]
                .reshape(ST, P, MG).transpose(1, 0, 2).reshape(P, ST * MG)
            ).astype(f8e4),
        })
    return in_maps


def assemble_output(results):
    out = np.empty((B, S, H), dtype=np.float32)
    for c in range(N_CORES):
        b, hg = divmod(c, 2)
        r = np.asarray(results[c]["outB"])  # [P, SC*MT*512] f16
        out[b, :, hg * MG:(hg + 1) * MG] = (
            r.reshape(P, SC, MT, 512).transpose(1, 3, 2, 0).reshape(S, MG)
        ).astype(np.float32)
    return out


def kernel(hidden_states, queries_weight):
    from concourse.bass_utils import run_bass_kernel_spmd

    in_maps = make_in_maps(hidden_states, queries_weight)
    res = run_bass_kernel_spmd(
        _get_nc(), in_maps, core_ids=list(range(N_CORES))
    ).results
    return assemble_output(res)


if __name__ == "__main__":
    x = np.random.randn(B, S, H).astype(np.float32)
    w = np.random.randn(H, H).astype(np.float32) * 1e-4
    out = kernel(x, w)
    print(out.shape, out.dtype)
